# revision 40
# baseline (speedup 1.0000x reference)
"""Multi-head attention (B=2, S=4096, D=512, H=8) on 8 TRN2 NeuronCores.

Sharding: core c handles batch c//4 and query rows (c%4)*1024..+1024; each
core runs full attention (all 8 heads) for its query block; host concats.

v5 design (per core):
  - All matmuls bf16 (fp8 paths were tried and rejected: per-element fp8
    error is multiplicative in the scores, and the jax inputs contain
    |s|~9.7 peaked rows where that distorts near-tied softmax weights).
  - Q/K/V projections streamed; K window 0 + the first Q group are
    emitted first so attention starts while the rest projects; the V
    projection is interleaved into head-window 0's score slices.
  - softmax 1/sqrt(dk) folded into the exp scale (1/8); b_k dropped
    (softmax-invariant); b_v folded into b_o host-side.
  - exp split per key-group between ACT (table exp) and DVE (Schraudolph
    bf16 bit-trick), interleaved for smooth dual-engine occupancy.
  - PV flipped: stationary = prob tile [128 keys, 128 q] bf16, moving =
    V|ones [128, 65] bf16 -> ctx [128 q, 64] + Z column in PSUM (Z is
    free).  Normalization is a per-partition scale (reciprocal of the Z
    column) fused into the PSUM->SBUF copy; the [q, dk] -> [dk, q]
    layout flip for the O-projection is a head-pair-batched SBUF->SBUF
    DMA transpose (XBAR), costing no engine time.
  - PV matmuls of head-window n-1 are interleaved 8-at-a-time between
    the score groups of head-window n; per-block tail ops are deferred a
    few slices so every engine finds its inputs ready at the queue head.
  - PSUM: score pool 3x2 banks (also used by the projections and the
    O-projection psum), ctx pool 2x1 banks.
"""

from contextlib import ExitStack

import numpy as np

import concourse.tile as tile
from concourse import bacc, mybir
from concourse.bass_utils import run_bass_kernel_spmd

D = 512
DK = 64
F32 = mybir.dt.float32
BF16 = mybir.dt.bfloat16
FP8 = mybir.dt.float8e4
I16 = mybir.dt.int16
F16 = mybir.dt.float16
EXP = mybir.ActivationFunctionType.Exp
IDENT = mybir.ActivationFunctionType.Identity
MULT = mybir.AluOpType.mult
ADD = mybir.AluOpType.add
SUB = mybir.AluOpType.subtract
DR = mybir.MatmulPerfMode.DoubleRow

SHIFT = -4.5
SSCALE = 1.0 / 8.0       # exp reads raw psum scores with scale 1/sqrt(dk)
SCHR_A8 = 184.662716 / 8.0
SCHR_B = 16256.0 - 5.5 + 0.5 + SHIFT * 184.662716

# exp engine per key-group: 'a' = ACT table exp, 'd' = DVE Schraudolph.
# PE-bound kernel -> bias toward ACT (table exp is exact; Schraudolph
# carries ~2-4% relative error) while keeping both engines under the PE
# ceiling.
ENG_PATTERN = "adadadadadadadad"   # 8 ACT / 8 DVE, interleaved


def build(T=1024, S=4096, n_cores=8, eng_pattern=ENG_PATTERN, **_unused):
    FC = D // 128   # feature chunks (contraction)
    SC = S // 128   # key chunks
    NG = SC // 2    # key groups of 256
    NW = T // 512   # query windows
    QW = 512
    KW = S // 512   # key windows (projection streaming)

    nc = bacc.Bacc("TRN2", target_bir_lowering=False, debug=False,
                   num_devices=n_cores)

    qT = nc.dram_tensor("qT", [D, T], F16, kind="ExternalInput").ap()
    kT8h = nc.dram_tensor("kT8h", [128, 2, 2, S], FP8,
                          kind="ExternalInput").ap()
    kT8l = nc.dram_tensor("kT8l", [128, 2, 2, S], FP8,
                          kind="ExternalInput").ap()
    vT8h = nc.dram_tensor("vT8h", [128, 2, 2, S], FP8,
                          kind="ExternalInput").ap()
    vT8l = nc.dram_tensor("vT8l", [128, 2, 2, S], FP8,
                          kind="ExternalInput").ap()
    wqT = nc.dram_tensor("wqT", [D, D], F16, kind="ExternalInput").ap()
    wk8h = nc.dram_tensor("wk8h", [128, 2, 2, D], FP8,
                          kind="ExternalInput").ap()
    wk8l = nc.dram_tensor("wk8l", [128, 2, 2, D], FP8,
                          kind="ExternalInput").ap()
    wv8h = nc.dram_tensor("wv8h", [128, 2, 2, D], FP8,
                          kind="ExternalInput").ap()
    wv8l = nc.dram_tensor("wv8l", [128, 2, 2, D], FP8,
                          kind="ExternalInput").ap()
    woT = nc.dram_tensor("woT", [D, D], BF16, kind="ExternalInput").ap()
    bq = nc.dram_tensor("bq", [D, 1], F32, kind="ExternalInput").ap()
    boe = nc.dram_tensor("boe", [1, D], F32, kind="ExternalInput").ap()
    y = nc.dram_tensor("y", [T, D], F32, kind="ExternalOutput").ap()

    with tile.TileContext(nc) as tc, ExitStack() as ctx:
        const = ctx.enter_context(tc.tile_pool(name="const", bufs=1))
        k8p = ctx.enter_context(tc.tile_pool(name="k8p", bufs=1))
        q8p = ctx.enter_context(tc.tile_pool(name="q8p", bufs=1))
        v8p = ctx.enter_context(tc.tile_pool(name="v8p", bufs=1))
        ctxsb = ctx.enter_context(tc.tile_pool(name="ctxsb", bufs=1))
        ptp = ctx.enter_context(tc.tile_pool(name="ptp", bufs=33))
        cn2p = ctx.enter_context(tc.tile_pool(name="cn2p", bufs=6))
        rzp = ctx.enter_context(tc.tile_pool(name="rzp", bufs=4))
        yp = ctx.enter_context(tc.tile_pool(name="yp", bufs=3))
        krawp = ctx.enter_context(tc.tile_pool(name="krawp", bufs=3))
        vrawp = ctx.enter_context(tc.tile_pool(name="vrawp", bufs=3))
        # PSUM: scorep 3x2 banks (also O-proj psum), ctxp 2x1 -> 8 banks
        scorep = ctx.enter_context(
            tc.tile_pool(name="scorep", bufs=3, space="PSUM"))
        ctxp = ctx.enter_context(
            tc.tile_pool(name="ctxp", bufs=2, space="PSUM"))

        # ---- constants ----
        wq3 = wqT.rearrange("(f p) d -> p f d", p=128)
        wq_c = const.tile([128, FC, D], F16, name="wq_c", tag="wq_c")
        qT3 = qT.rearrange("(f p) t -> p f t", p=128)
        qraw_c = const.tile([128, FC, T], F16, name="qraw_c", tag="qraw_c")
        nc.sync.dma_start(wq_c[:, :, 0:256], wq3[:, :, 0:256])
        nc.sync.dma_start(qraw_c[:, :, 0:QW], qT3[:, :, 0:QW])
        wkh_c = const.tile([128, 2, 2, D], FP8, name="wkh_c", tag="wkh_c")
        nc.scalar.dma_start(wkh_c[:], wk8h[:])
        wkl_c = const.tile([128, 2, 2, D], FP8, name="wkl_c", tag="wkl_c")
        nc.scalar.dma_start(wkl_c[:], wk8l[:])
        wvh_c = const.tile([128, 2, 2, D], FP8, name="wvh_c", tag="wvh_c")
        nc.scalar.dma_start(wvh_c[:], wv8h[:])
        wvl_c = const.tile([128, 2, 2, D], FP8, name="wvl_c", tag="wvl_c")
        nc.scalar.dma_start(wvl_c[:], wv8l[:])
        wo_c = const.tile([128, FC, D], BF16, name="wo_c", tag="wo_c")
        nc.scalar.dma_start(wo_c[:], woT.rearrange("(f p) d -> p f d", p=128))
        wq_t = [wq_c[:, f, :] for f in range(FC)]
        wo_t = [wo_c[:, f, :] for f in range(FC)]
        qraw = [qraw_c[:, f, :] for f in range(FC)]
        bq_c = const.tile([128, FC, 1], F32, name="bq_c", tag="bq_c")
        nc.sync.dma_start(bq_c[:], bq.rearrange("(f p) o -> p f o", p=128))
        bq_t = [bq_c[:, f, :] for f in range(FC)]
        ebias = const.tile([128, 1], F32, name="ebias", tag="ebias")
        nc.vector.memset(ebias[:], SHIFT)
        zbias = const.tile([128, 1], F32, name="zbias", tag="zbias")
        nc.vector.memset(zbias[:], 0.0)
        boe_row = const.tile([1, D], F32, name="boe_row", tag="boe_row")
        nc.sync.dma_start(boe_row[:], boe[:])
        bo_bc = const.tile([128, D], F32, name="bo_bc", tag="bo_bc")
        nc.gpsimd.partition_broadcast(bo_bc[:], boe_row[0:1, :])

        # ---- persistent activation tensors ----
        # kt[hc][w8]: [128, 512] bf16; partitions = head-pair dk rows
        kt = [[k8p.tile([128, 512], F16, name=f"kt_{hc}_{w}",
                        tag=f"kt_{hc}_{w}") for w in range(KW)]
              for hc in range(FC)]
        qb = [[q8p.tile([128, QW], F16, name=f"qb_{hc}_{w}",
                        tag=f"qb_{hc}_{w}") for w in range(NW)]
              for hc in range(FC)]
        # v8[g]: [128 keys, 2 (chunk parity), 8 heads, 65] bf16 (V | ones)
        v8 = [v8p.tile([128, 2, 8, 65], BF16, name=f"v8_{g}", tag=f"v8_{g}")
              for g in range(NG)]
        # CTX[f]: [128, T] bf16, partitions = d_model rows 128f..128f+127
        CTX = [ctxsb.tile([128, T], BF16, name=f"CTX{f}", tag=f"CTX{f}")
               for f in range(FC)]

        for g in range(NG):
            nc.gpsimd.memset(v8[g][:, :, :, 64:65], 1.0)

        # ---- projections (streamed; attention overlaps via tile deps) ----
        def q_proj_group(w, hc2):
            ps = scorep.tile([128, 2, QW], F32, name="psq", tag="sc")
            for j, hc in enumerate((hc2, hc2 + 1)):
                for f in range(FC):
                    nc.tensor.matmul(
                        ps[:, j, :],
                        wq_t[f][:, hc * 128:(hc + 1) * 128],
                        qraw[f][:, w * QW:(w + 1) * QW],
                        start=(f == 0), stop=(f == FC - 1))
            for j, hc in enumerate((hc2, hc2 + 1)):
                nc.scalar.activation(
                    qb[hc][w][:], ps[:, j, :], IDENT,
                    bias=bq_t[hc][:], scale=1.0)

        vraw_tiles = {}

        def k_proj_window(w8):
            wsl = slice(w8 * 512, (w8 + 1) * 512)
            kh = krawp.tile([128, 2, 2, 512], FP8, name="krh", tag="krh")
            nc.sync.dma_start(kh[:], kT8h[:, :, :, wsl])
            kl = krawp.tile([128, 2, 2, 512], FP8, name="krl", tag="krl")
            nc.sync.dma_start(kl[:], kT8l[:, :, :, wsl])
            for hc2 in range(0, FC, 2):
                ps = scorep.tile([128, 2, 512], F32, name="psk", tag="sc")
                for j, hc in enumerate((hc2, hc2 + 1)):
                    hsl = slice(hc * 128, (hc + 1) * 128)
                    passes = [(kh, wkh_c), (kh, wkl_c), (kl, wkh_c)]
                    for pi, (xa, wa) in enumerate(passes):
                        for gg in range(2):
                            nc.tensor.matmul(
                                ps[:, j, :],
                                wa[:, gg, :, hsl],
                                xa[:, gg, :, :],
                                start=(pi == 0 and gg == 0),
                                stop=(pi == 2 and gg == 1), perf_mode=DR)
                for j, hc in enumerate((hc2, hc2 + 1)):
                    nc.scalar.activation(kt[hc][w8][:], ps[:, j, :], IDENT,
                                         bias=zbias[:], scale=1.0 / 16.0)

        def v_proj_group(g):
            # V chunks 2g, 2g+1 -> v8[g]; one batched copy.  The window's
            # vraw DMA is issued just-in-time at its first group.
            w8 = g // 2
            wsl = slice(w8 * 512, (w8 + 1) * 512)
            if g % 2 == 0:
                vh = vrawp.tile([128, 2, 2, 512], FP8, name="vrh", tag="vrh")
                nc.scalar.dma_start(vh[:], vT8h[:, :, :, wsl])
                vl = vrawp.tile([128, 2, 2, 512], FP8, name="vrl", tag="vrl")
                nc.scalar.dma_start(vl[:], vT8l[:, :, :, wsl])
                vraw_tiles[w8] = (vh, vl)
            vh, vl = vraw_tiles[w8]
            ps = scorep.tile([128, 2, 512], F32, name="psv", tag="sc")
            for j in range(2):
                kc = 2 * (g % 2) + j
                ksl = slice(kc * 128, (kc + 1) * 128)
                passes = [(vh, wvh_c), (vh, wvl_c), (vl, wvh_c)]
                for pi, (xa, wa) in enumerate(passes):
                    for gg in range(2):
                        nc.tensor.matmul(
                            ps[:, j, :],
                            xa[:, gg, :, ksl],
                            wa[:, gg, :, :],
                            start=(pi == 0 and gg == 0),
                            stop=(pi == 2 and gg == 1), perf_mode=DR)
            nc.scalar.activation(
                v8[g][:, :, :, 0:DK],
                ps.rearrange("p j (h d) -> p j h d", d=DK)[:],
                IDENT, bias=zbias[:], scale=1.0 / 16.0)

        # first Q-proj group (covers heads 0-3 of both hc 0/1) and K window
        # 0 come first so head-window 0 can start ASAP; the bulk wq/qraw
        # loads are queued behind kraw window 0.
        q_proj_group(0, 0)
        k_proj_window(0)
        nc.sync.dma_start(wq_c[:, :, 256:D], wq3[:, :, 256:D])
        nc.sync.dma_start(qraw_c[:, :, QW:T], qT3[:, :, QW:T])
        q_proj_group(0, 2)
        for hc2 in range(0, FC, 2):
            q_proj_group(1, hc2)
        for w8 in range(1, KW):
            k_proj_window(w8)

        # ---- attention ----
        hws = [(w, h) for w in range(NW) for h in range(8)]
        pt_tiles = {}
        ct_tiles = {}
        cn2_tiles = {}

        def emit_scores_group(n, g):
            w, h = hws[n]
            hc, e = h // 2, h % 2
            sp = scorep.tile([128, 2, QW], F32, name="sp", tag="sc")
            mv = qb[hc][w][64 * e:64 * e + 64, :]
            for ci in range(2):
                c = 2 * g + ci
                st = kt[hc][c // 4][64 * e:64 * e + 64,
                                    (c % 4) * 128:(c % 4) * 128 + 128]
                nc.tensor.matmul(sp[:, ci, :], st, mv,
                                 start=True, stop=True)
            pt = ptp.tile([128, 2, QW], BF16, name="pt", tag="pt")
            if eng_pattern[g % len(eng_pattern)] == "a":
                nc.scalar.activation(pt[:], sp[:], EXP, bias=ebias[:],
                                     scale=SSCALE)
            else:
                nc.vector.tensor_scalar(
                    pt.bitcast(I16)[:], sp[:], SCHR_A8, SCHR_B, MULT, ADD)
            pt_tiles[(n % 2, g)] = pt

        pending = []   # (due_slice, seq, fn) -- deferred tail ops

        def defer(due, fn):
            pending.append((due, len(pending), fn))

        def flush(t):
            pending.sort()
            while pending and pending[0][0] <= t:
                pending.pop(0)[2]()

        def emit_pv_slice(m, g, t):
            # at score-group g of hw m+1: block j = g//4, chunks 8*(g%4)..+8
            w, h = hws[m]
            hc, e = h // 2, h % 2
            j = g // 4
            c0 = 8 * (g % 4)
            if c0 == 0:
                ct_tiles[(m % 2, j % 2)] = ctxp.tile(
                    [128, 512], F32, name="ct", tag="ct")
            ct = ct_tiles[(m % 2, j % 2)]
            for c in range(c0, c0 + 8):
                pt = pt_tiles[(m % 2, c // 2)]
                nc.tensor.matmul(
                    ct[:, 0:65],
                    pt[:, c % 2, j * 128:(j + 1) * 128],
                    v8[c // 2][:, c % 2, h, :],
                    start=(c == 0), stop=(c == SC - 1))
            if c0 == 24:
                # same-slice: reciprocal of the Z column (DVE, input just
                # closed by PE).  Deferred: normalize into the head-pair
                # staging tile (ACT), then DMA-transpose both heads' halves
                # into CTX once the pair is complete.
                rz = rzp.tile([128, 1], F32, name="rz", tag="rz")
                nc.vector.reciprocal(rz[:], ct[:, 64:65])
                if e == 0:
                    cn2_tiles[(hc, w, j)] = cn2p.tile(
                        [128, 128], BF16, name="cn2", tag="cn2")
                cn2 = cn2_tiles[(hc, w, j)]

                def norm(ct=ct, rz=rz, cn2=cn2, e=e):
                    nc.scalar.activation(
                        cn2[:, 64 * e:64 * e + 64], ct[:, 0:DK], IDENT,
                        bias=zbias[:], scale=rz[:])

                defer(t + 2, norm)
                if e == 1:
                    def ctr(cn2=cn2, hc=hc, w=w, j=j):
                        nc.sync.dma_start(
                            CTX[hc][:, w * QW + j * 128:w * QW + j * 128
                                    + 128],
                            cn2[:], transpose=True)
                    defer(t + 3, ctr)

        def emit_oproj(w, ti):
            sp = scorep.tile([128, 2, QW], F32, name="spy", tag="sc")
            ps_y = sp[:, 0, :]
            col = w * QW + ti * 128
            for f in range(FC):
                nc.tensor.matmul(
                    ps_y, CTX[f][:, col:col + 128], wo_t[f][:],
                    start=(f == 0), stop=(f == FC - 1))
            yt = yp.tile([128, D], F32, name="yt", tag="y")
            nc.vector.scalar_tensor_tensor(
                yt[:], ps_y, 1.0, bo_bc[:], MULT, ADD)
            eng = nc.sync if ti % 2 == 0 else nc.scalar
            eng.dma_start(y[col:col + 128, :], yt[:])

        for n in range(len(hws) + 1):
            for g in range(NG):
                t = n * NG + g
                flush(t)
                if n == 0:
                    v_proj_group(g)
                if n > 0:
                    emit_pv_slice(n - 1, g, t)
                    if g == NG - 1:
                        wm, hm = hws[n - 1]
                        if hm == 7:
                            for ti in range(4):
                                defer(t + 9 + 2 * ti,
                                      lambda w=wm, ti=ti: emit_oproj(w, ti))
                if n < len(hws):
                    emit_scores_group(n, g)
        flush(10 ** 9)

    nc.compile()
    return nc


_CACHE = {}


def _get_compiled():
    if "nc" not in _CACHE:
        _CACHE["nc"] = build(T=1024, S=4096, n_cores=8)
    return _CACHE["nc"]


def make_in_maps(q, k, v, W_q, b_q, W_k, b_k, W_v, b_v, W_o, b_o, n_cores=8):
    import ml_dtypes
    bf = ml_dtypes.bfloat16
    f = np.float32
    f16 = np.float16
    f8 = ml_dtypes.float8_e4m3

    def pair_hilo(xT, scale=1.0):
        # xT [D, N] -> hi/lo fp8 pair layouts [128, 2, 2, N]:
        # x8[k, g, i, n] = xT[256g + 128i + k, n]
        Dd, N = xT.shape
        x = (xT * scale).reshape(2, 2, 128, N).transpose(2, 0, 1, 3)
        hi = x.astype(f8)
        lo = (x - hi.astype(f)).astype(f8)
        return np.ascontiguousarray(hi), np.ascontiguousarray(lo)

    qT = [np.ascontiguousarray(np.asarray(q[b], f).T.astype(f16))
          for b in range(q.shape[0])]
    kTl = [pair_hilo(np.asarray(k[b], f).T) for b in range(k.shape[0])]
    vTl = [pair_hilo(np.asarray(v[b], f).T) for b in range(v.shape[0])]
    bo_eff = np.asarray(b_o, f) + np.asarray(W_o, f) @ np.asarray(b_v, f)
    shared = {
        "wqT": np.ascontiguousarray(np.asarray(W_q, f).T.astype(f16)),
        "wk8h": None, "wk8l": None,
        "wv8h": None, "wv8l": None,
        "woT": np.ascontiguousarray(np.asarray(W_o, f).T.astype(bf)),
        "bq": np.asarray(b_q, f).reshape(D, 1),
        "boe": bo_eff.reshape(1, D).astype(f),
    }
    wvh, wvl = pair_hilo(np.asarray(W_v, f).T, scale=16.0)
    shared["wv8h"], shared["wv8l"] = wvh, wvl
    wkh, wkl = pair_hilo(np.asarray(W_k, f).T, scale=16.0)
    shared["wk8h"], shared["wk8l"] = wkh, wkl
    n_b = q.shape[0]
    blocks_per_b = n_cores // n_b
    T = q.shape[1] // blocks_per_b
    in_maps = []
    for c in range(n_cores):
        b, wdx = divmod(c, blocks_per_b)
        m = dict(shared)
        m["qT"] = np.ascontiguousarray(qT[b][:, wdx * T:(wdx + 1) * T])
        m["kT8h"], m["kT8l"] = kTl[b]
        m["vT8h"], m["vT8l"] = vTl[b]
        in_maps.append(m)
    return in_maps


def kernel(q, k, v, W_q, b_q, W_k, b_k, W_v, b_v, W_o, b_o):
    nc = _get_compiled()
    in_maps = make_in_maps(q, k, v, W_q, b_q, W_k, b_k, W_v, b_v, W_o, b_o)
    res = run_bass_kernel_spmd(nc, in_maps, list(range(8)))
    B, S_full = q.shape[0], q.shape[1]
    T = S_full // (8 // B)
    out = np.empty((B, S_full, D), np.float32)
    for c in range(8):
        b, wdx = divmod(c, 8 // B)
        out[b, wdx * T:(wdx + 1) * T, :] = res.results[c]["y"]
    return out
